# revision 13
# baseline (speedup 1.0000x reference)
"""ConformerBlock Trainium2 Bass kernel.

Sharding: data-parallel over batch (B=8) across the 8 NeuronCores; all
weights replicated per core; no collectives.

Per-core layout: feature-major residual stream xT [D=512, S=1024] in SBUF
as a [128, 4, 1024] tile. All matmuls run in float32r (TF32-like, ~1e-4
rel error, full PE rate at N=512). Host-side weight prep:
  - LayerNorm gamma/beta folded into the following linear's weights/bias
  - the 0.5 FFN scale folded into the FFN output weights
  - BatchNorm folded into the depthwise-conv weights/bias
  - depthwise grouped conv lowered to 31 taps x 8 channel-chunks of
    block-diagonal [128, 128] matrices applied as shifted accumulating
    matmuls in PSUM.
LN statistics use an all-ones [128,128] lhsT matmul, which yields the
per-token sums broadcast across all 128 partitions for free; softmax
normalization is deferred past the attention-value matmul (the value
matrix carries an extra ones column so PSUM row 64 accumulates the
softmax denominators), then applied via a K=1 broadcast matmul.
"""
import sys

sys.path.insert(0, '/opt/trn_rl_repo')

import numpy as np

import concourse.tile as tile
from concourse import bacc, mybir

F32 = mybir.dt.float32
F32R = mybir.dt.float32r
AF = mybir.ActivationFunctionType
ALU = mybir.AluOpType

D = 512            # model dim
S = 1024           # sequence length
B = 8              # batch (one element per core)
HEADS = 8
DH = 64            # head dim
FF_HID = 2048      # ffn hidden (per GLU half)
CONV_IN = 1024     # conv inner dim (per GLU half)
KER = 31
PAD = 15
LN_EPS = 1e-5
BN_EPS = 1e-5
TC = 2             # token chunks
TN = 512           # tokens per chunk
FC = 4             # feature chunks of D

_CACHE = {}


# --------------------------------------------------------------------------
# host-side weight preparation
# --------------------------------------------------------------------------

def _prep(inputs):
    g = {k: np.asarray(v, dtype=np.float64) for k, v in inputs.items()}
    p = {}

    def lin(w, ln_g, ln_b, bias):
        # y = ln_out @ W.T + bias with ln_out = xhat*g + b  =>
        # y = xhat @ (W*g).T + (W @ b + bias)
        return w * ln_g[None, :], w @ ln_b + bias

    for tag in ("ff1", "ff2"):
        w_in, b_in = lin(g[f"{tag}_win"], g[f"{tag}_g"], g[f"{tag}_b"],
                         g[f"{tag}_bin"])
        p[f"{tag}_win_t"] = np.ascontiguousarray(w_in.T, np.float32)         # [512, 4096]
        p[f"{tag}_bin"] = b_in.astype(np.float32)
        p[f"{tag}_wout_t"] = np.ascontiguousarray((0.5 * g[f"{tag}_wout"]).T,
                                                  np.float32)                # [2048, 512]
        p[f"{tag}_bout"] = (0.5 * g[f"{tag}_bout"]).astype(np.float32)

    wqkv, bqkv = lin(g["wqkv"], g["attn_g"], g["attn_b"], g["bqkv"])
    p["wqkv_t"] = np.ascontiguousarray(wqkv.T, np.float32)                   # [512, 1536]
    p["bqkv"] = bqkv.astype(np.float32)
    p["wo_t"] = np.ascontiguousarray(g["wo"].T, np.float32)                  # [512, 512]
    p["bo"] = g["bo"].astype(np.float32)

    pw1, pw1b = lin(g["pw1_w"][:, :, 0], g["conv_g"], g["conv_b"], g["pw1_b"])
    p["pw1_t"] = np.ascontiguousarray(pw1.T, np.float32)                     # [512, 2048]
    p["pw1_b"] = pw1b.astype(np.float32)

    # depthwise conv + BN fold -> block-diag matrices [8 cc][31 t][128 ki, 128 mo]
    scale = g["bn_g"] / np.sqrt(g["bn_v"] + BN_EPS)                          # [1024]
    dw = g["dw_w"] * scale[:, None, None]                                    # [1024, 8, 31]
    dwb = (g["dw_b"] - g["bn_m"]) * scale + g["bn_b"]                        # [1024]
    dwm = np.zeros((8, KER, 128, 128), np.float64)
    o = np.arange(128)
    grp = (o // 8) * 8
    for j in range(8):
        ki = grp + j
        for cc in range(8):
            dwm[cc, :, ki, o] = dw[cc * 128 + o, j, :]   # LHS adv-index -> [128, 31]
    p["dwm"] = np.ascontiguousarray(dwm, np.float32)
    p["dw_b"] = dwb.astype(np.float32)

    p["pw2_t"] = np.ascontiguousarray(g["pw2_w"][:, :, 0].T, np.float32)     # [1024, 512]
    p["pw2_b"] = g["pw2_b"].astype(np.float32)

    p["fn_g"] = g["fn_g"].astype(np.float32)
    p["fn_b"] = g["fn_b"].astype(np.float32)

    p["ones"] = np.ones((128, 128), np.float32)
    p["onesrow"] = np.ones((1, TN), np.float32)
    return p


# --------------------------------------------------------------------------
# device program
# --------------------------------------------------------------------------

def _build(bias_nz, debug=False, nreps=1):
    """bias_nz: dict name->bool; device-side bias adds are emitted only for
    entries that are True (the rest are exactly zero in the input arrays)."""
    nc = bacc.Bacc("TRN2", target_bir_lowering=False, debug=False)

    d = {}
    d["x"] = nc.dram_tensor("x", [D, S], F32R, kind="ExternalInput").ap()
    d["ones"] = nc.dram_tensor("ones", [128, 128], F32R, kind="ExternalInput").ap()
    d["onesrow"] = nc.dram_tensor("onesrow", [1, TN], F32R, kind="ExternalInput").ap()
    for tag in ("ff1", "ff2"):
        d[f"{tag}_win_t"] = nc.dram_tensor(f"{tag}_win_t", [D, 2 * FF_HID], F32R,
                                           kind="ExternalInput").ap()
        d[f"{tag}_wout_t"] = nc.dram_tensor(f"{tag}_wout_t", [FF_HID, D], F32R,
                                            kind="ExternalInput").ap()
        d[f"{tag}_bin"] = nc.dram_tensor(f"{tag}_bin", [2 * FF_HID], F32R,
                                         kind="ExternalInput").ap()
        d[f"{tag}_bout"] = nc.dram_tensor(f"{tag}_bout", [D], F32R,
                                          kind="ExternalInput").ap()
    d["wqkv_t"] = nc.dram_tensor("wqkv_t", [D, 3 * D], F32R, kind="ExternalInput").ap()
    d["bqkv"] = nc.dram_tensor("bqkv", [3 * D], F32R, kind="ExternalInput").ap()
    d["wo_t"] = nc.dram_tensor("wo_t", [D, D], F32R, kind="ExternalInput").ap()
    d["bo"] = nc.dram_tensor("bo", [D], F32R, kind="ExternalInput").ap()
    d["pw1_t"] = nc.dram_tensor("pw1_t", [D, 2 * CONV_IN], F32R,
                                kind="ExternalInput").ap()
    d["pw1_b"] = nc.dram_tensor("pw1_b", [2 * CONV_IN], F32R, kind="ExternalInput").ap()
    d["dwm"] = nc.dram_tensor("dwm", [8, KER, 128, 128], F32R,
                              kind="ExternalInput").ap()
    d["dw_b"] = nc.dram_tensor("dw_b", [CONV_IN], F32R, kind="ExternalInput").ap()
    d["pw2_t"] = nc.dram_tensor("pw2_t", [CONV_IN, D], F32R, kind="ExternalInput").ap()
    d["pw2_b"] = nc.dram_tensor("pw2_b", [D], F32R, kind="ExternalInput").ap()
    d["fn_g"] = nc.dram_tensor("fn_g", [D], F32, kind="ExternalInput").ap()
    d["fn_b"] = nc.dram_tensor("fn_b", [D], F32, kind="ExternalInput").ap()
    d["out"] = nc.dram_tensor("out", [D, S], F32, kind="ExternalOutput").ap()
    if debug:
        for i in range(1, 5):
            d[f"dbg{i}"] = nc.dram_tensor(f"dbg{i}", [D, S], F32,
                                          kind="ExternalOutput").ap()
        d["dbgh"] = nc.dram_tensor("dbgh", [D, S], F32, kind="ExternalOutput").ap()

    from contextlib import ExitStack
    with tile.TileContext(nc) as tc, ExitStack() as ctx:
        cpool = ctx.enter_context(tc.tile_pool(name="cpool", bufs=1))
        spool = ctx.enter_context(tc.tile_pool(name="spool", bufs=1))
        bp = ctx.enter_context(tc.tile_pool(name="bp", bufs=1))
        ps_acc = ctx.enter_context(tc.tile_pool(name="ps_acc", bufs=4, space="PSUM"))
        ps_s = ctx.enter_context(tc.tile_pool(name="ps_s", bufs=3, space="PSUM"))

        ones = cpool.tile([128, 128], F32R)
        nc.sync.dma_start(ones[:], d["ones"])

        any_bias = any(v for k, v in bias_nz.items() if k != "fn")
        onesrow = None
        if any_bias:
            onesrow = cpool.tile([1, TN], F32R, tag="onesrow")
            nc.sync.dma_start(onesrow[:], d["onesrow"])

        xs = spool.tile([128, FC, S], F32R)
        nc.sync.dma_start(xs[:], d["x"].rearrange("(c p) n -> p c n", p=128))

        brow = {}
        for name, width in (("ff1_bin", 2 * FF_HID), ("ff1_bout", D),
                            ("ff2_bin", 2 * FF_HID), ("ff2_bout", D),
                            ("bqkv", 3 * D), ("bo", D), ("pw1_b", 2 * CONV_IN),
                            ("dw_b", CONV_IN), ("pw2_b", D)):
            if bias_nz.get(name):
                t = bp.tile([1, width], F32R, tag=name)
                nc.sync.dma_start(t[:], d[name][None, :])
                brow[name] = t

        def accum(acc, terms, bias=None, n=TN):
            """PSUM accumulation: matmuls over `terms` [(lhsT, rhs)...] plus an
            optional per-partition bias via a K=1 matmul; sets start/stop."""
            last = len(terms) - 1 + (1 if bias else 0)
            i = 0
            for l, r in terms:
                nc.tensor.matmul(acc[:], l, r, start=(i == 0), stop=(i == last))
                i += 1
            if bias:
                name, mo = bias
                nc.tensor.matmul(acc[:],
                                 brow[name][:, mo * 128:(mo + 1) * 128],
                                 onesrow[:, 0:n],
                                 start=False, stop=True)

        # ------------------------------------------------------------------
        def layer_norm(h_out, lnp):
            for t in range(TC):
                sl = slice(t * TN, (t + 1) * TN)
                bc_s = ps_s.tile([128, TN], F32, tag="s")
                accum(bc_s, [(ones[:], xs[:, c, sl]) for c in range(FC)])
                xsq = lnp.tile([128, FC, TN], F32R, tag="xsq")
                for c in range(FC):
                    nc.scalar.activation(xsq[:, c, :], xs[:, c, sl].bitcast(F32),
                                         AF.Square)
                bc_q = ps_s.tile([128, TN], F32, tag="s")
                accum(bc_q, [(ones[:], xsq[:, c, :]) for c in range(FC)])
                mu = lnp.tile([128, TN], F32, tag="mu")
                nc.vector.tensor_scalar(mu[:], bc_s[:], 1.0 / D, None, ALU.mult)
                ve = lnp.tile([128, TN], F32, tag="ve")
                nc.vector.tensor_scalar(ve[:], bc_q[:], 1.0 / D, LN_EPS,
                                        ALU.mult, ALU.add)
                m2 = lnp.tile([128, TN], F32, tag="m2")
                nc.vector.tensor_tensor(m2[:], mu[:], mu[:], ALU.mult)
                nc.vector.tensor_tensor(ve[:], ve[:], m2[:], ALU.subtract)
                nc.scalar.activation(m2[:], ve[:], AF.Ln)
                rsig = lnp.tile([128, TN], F32, tag="rsig")
                nc.scalar.activation(rsig[:], m2[:], AF.Exp, scale=-0.5)
                for c in range(FC):
                    dd = lnp.tile([128, TN], F32, tag="dd")
                    nc.vector.tensor_tensor(dd[:], xs[:, c, sl].bitcast(F32),
                                            mu[:], ALU.subtract)
                    nc.vector.tensor_tensor(h_out[:, c, sl], dd[:], rsig[:],
                                            ALU.mult)

        # ------------------------------------------------------------------
        def ffn(tag, pools, dbg=False):
            lnp, hpool, wff, fsp = pools
            h = hpool.tile([128, FC, S], F32R, tag="h")
            layer_norm(h, lnp)
            if dbg:
                nc.sync.dma_start(d["dbgh"].rearrange("(c p) n -> p c n", p=128),
                                  h[:].bitcast(F32))
            w_in = wff.tile([128, FC, 2 * FF_HID], F32R, tag="wffin")
            nc.sync.dma_start(w_in[:],
                              d[f"{tag}_win_t"].rearrange("(c p) m -> p c m", p=128))
            nh = FF_HID // 128  # 16
            w_out = wff.tile([128, nh, D], F32R, tag="wffout")
            nc.sync.dma_start(w_out[:],
                              d[f"{tag}_wout_t"].rearrange("(c p) m -> p c m", p=128))
            b_in = f"{tag}_bin"
            b_out = f"{tag}_bout"
            for t in range(TC):
                sl = slice(t * TN, (t + 1) * TN)
                accs = []
                for m in range(FC):
                    acc_t = ps_acc.tile([128, TN], F32, tag="acc")
                    accs.append(acc_t)
                for hc in range(nh):
                    a_ps = ps_s.tile([128, TN], F32, tag="s")
                    accum(a_ps,
                          [(w_in[:, c, hc * 128:(hc + 1) * 128], h[:, c, sl])
                           for c in range(FC)],
                          bias=(b_in, hc) if bias_nz[b_in] else None)
                    c_ps = ps_s.tile([128, TN], F32, tag="s")
                    accum(c_ps,
                          [(w_in[:, c, (nh + hc) * 128:(nh + hc + 1) * 128],
                            h[:, c, sl]) for c in range(FC)],
                          bias=(b_in, nh + hc) if bias_nz[b_in] else None)
                    a_sb = fsp.tile([128, TN], F32, tag="asb")
                    nc.scalar.activation(a_sb[:], a_ps[:], AF.Silu)
                    g_sb = fsp.tile([128, TN], F32R, tag="gsb")
                    nc.vector.tensor_tensor(g_sb[:], a_sb[:], c_ps[:], ALU.mult)
                    for m in range(FC):
                        is_last = hc == nh - 1 and not bias_nz[b_out]
                        nc.tensor.matmul(accs[m][:],
                                         w_out[:, hc, m * 128:(m + 1) * 128],
                                         g_sb[:],
                                         start=(hc == 0), stop=is_last)
                for m in range(FC):
                    if bias_nz[b_out]:
                        nc.tensor.matmul(accs[m][:],
                                         brow[b_out][:, m * 128:(m + 1) * 128],
                                         onesrow[:, 0:TN], start=False, stop=True)
                    nc.vector.tensor_tensor(xs[:, m, sl], accs[m][:],
                                            xs[:, m, sl].bitcast(F32), ALU.add)

        # ------------------------------------------------------------------
        def attention(pools):
            lnp, hpool, watt, attp, att2, att3 = pools
            h = hpool.tile([128, FC, S], F32R, tag="h")
            layer_norm(h, lnp)
            wqkv = watt.tile([128, FC, 3 * D], F32R, tag="wqkv")
            nc.sync.dma_start(wqkv[:],
                              d["wqkv_t"].rearrange("(c p) m -> p c m", p=128))
            wo = watt.tile([128, FC, D], F32R, tag="wo")
            nc.sync.dma_start(wo[:], d["wo_t"].rearrange("(c p) m -> p c m", p=128))
            bvq = None
            if bias_nz["bqkv"]:
                bvq = bp.tile([128, FC], F32, tag="bvq")
                nc.sync.dma_start(bvq[:],
                                  d["bqkv"][2 * D:3 * D].rearrange("(c p) -> p c",
                                                                   p=128))

            q_sb = attp.tile([128, FC, S], F32R, tag="q")
            k_sb = attp.tile([128, FC, S], F32R, tag="k")
            for fc in range(FC):
                for t in range(TC):
                    sl = slice(t * TN, (t + 1) * TN)
                    for which, base in (("q", 0), ("k", D)):
                        pp = ps_s.tile([128, TN], F32, tag="s")
                        mo = base // 128 + fc
                        accum(pp,
                              [(wqkv[:, c, mo * 128:(mo + 1) * 128], h[:, c, sl])
                               for c in range(FC)],
                              bias=("bqkv", mo) if bias_nz["bqkv"] else None)
                        dst = q_sb if which == "q" else k_sb
                        nc.scalar.activation(dst[:, fc, sl], pp[:], AF.Copy)

            # v token-major with ones column at index 64 per head
            vaug = attp.tile([128, 8, HEADS, 66], F32R, tag="vaug")
            nc.sync.dma_start(
                vaug[:, :, :, 64:65],
                d["ones"][:, 0:64].rearrange("p (a b) -> p a b", a=8).unsqueeze(3))
            for kc in range(8):
                v_ps = ps_s.tile([128, D], F32, tag="s")
                terms = [(h[:, c, kc * 128:(kc + 1) * 128], wqkv[:, c, 2 * D:3 * D])
                         for c in range(FC)]
                last = len(terms) - 1 + (1 if bias_nz["bqkv"] else 0)
                for i, (l, r) in enumerate(terms):
                    nc.tensor.matmul(v_ps[:], l, r, start=(i == 0),
                                     stop=(i == last))
                if bias_nz["bqkv"]:
                    nc.tensor.matmul(v_ps[:], ones[0:1, :],
                                     brow["bqkv"][:, 2 * D:3 * D],
                                     start=False, stop=True)
                nc.scalar.activation(
                    vaug[:, kc, :, 0:64],
                    v_ps[:].rearrange("p (h e) -> p h e", h=HEADS), AF.Copy)

            o_fm = attp.tile([128, FC, S], F32R, tag="ofm")
            for hd in range(HEADS):
                hb = (hd % 2) * 64
                hc = hd // 2
                for t in range(TC):
                    sl = slice(t * TN, (t + 1) * TN)
                    e_sb = att2.tile([128, 8, TN], F32R, tag="esb")
                    o_ps = ps_acc.tile([65, TN], F32, tag="acc")
                    for kc in range(8):
                        s_ps = ps_s.tile([128, TN], F32, tag="s")
                        nc.tensor.matmul(s_ps[:],
                                         k_sb[hb:hb + 64, hc, kc * 128:(kc + 1) * 128],
                                         q_sb[hb:hb + 64, hc, sl],
                                         start=True, stop=True)
                        nc.scalar.activation(e_sb[:, kc, :], s_ps[:], AF.Exp,
                                             scale=float(DH) ** -0.5)
                        nc.tensor.matmul(o_ps[:],
                                         vaug[:, kc, hd, 0:65],
                                         e_sb[:, kc, :],
                                         start=(kc == 0), stop=(kc == 7))
                    rows = att3.tile([1, 3, TN], F32, tag="rows")
                    nc.scalar.activation(rows[:, 2, :], o_ps[64:65, :], AF.Copy)
                    nc.vector.reciprocal_approx_accurate(
                        rows[:, 0, :], rows[:, 2, :], rows[:, 1, :])
                    rrow_r = att3.tile([1, TN], F32R, tag="rrowr")
                    nc.scalar.activation(rrow_r[:], rows[:, 0, :], AF.Copy)
                    bc_ps = ps_s.tile([64, TN], F32, tag="s")
                    nc.tensor.matmul(bc_ps[:], ones[0:1, 0:64], rrow_r[:],
                                     start=True, stop=True)
                    bc_sb = att3.tile([64, TN], F32, tag="bcsb")
                    nc.scalar.activation(bc_sb[:], bc_ps[:], AF.Copy)
                    nc.vector.tensor_tensor(o_fm[hb:hb + 64, hc, sl],
                                            o_ps[0:64, :], bc_sb[:], ALU.mult)
                    if bias_nz["bqkv"]:
                        # + v bias (softmax weights sum to one)
                        nc.vector.tensor_scalar(
                            o_fm[hb:hb + 64, hc, sl],
                            o_fm[hb:hb + 64, hc, sl].bitcast(F32),
                            bvq[hb:hb + 64, hc:hc + 1], None, ALU.add)

            for t in range(TC):
                sl = slice(t * TN, (t + 1) * TN)
                for m in range(FC):
                    acc = ps_acc.tile([128, TN], F32, tag="acc")
                    accum(acc,
                          [(wo[:, c, m * 128:(m + 1) * 128], o_fm[:, c, sl])
                           for c in range(FC)],
                          bias=("bo", m) if bias_nz["bo"] else None)
                    nc.vector.tensor_tensor(xs[:, m, sl], acc[:],
                                            xs[:, m, sl].bitcast(F32), ALU.add)

        # ------------------------------------------------------------------
        def conv(pools):
            lnp, hpool, wconv, wdw, convp, up, fsp = pools
            h = hpool.tile([128, FC, S], F32R, tag="h")
            layer_norm(h, lnp)
            pw1 = wconv.tile([128, FC, 2 * CONV_IN], F32R, tag="pw1")
            nc.sync.dma_start(pw1[:],
                              d["pw1_t"].rearrange("(c p) m -> p c m", p=128))
            ncc = CONV_IN // 128  # 8
            pw2 = wconv.tile([128, ncc, D], F32R, tag="pw2")
            nc.sync.dma_start(pw2[:],
                              d["pw2_t"].rearrange("(c p) m -> p c m", p=128))

            dvo = convp.tile([128, ncc, S], F32R, tag="dvo")
            for cc in range(ncc):
                u = up.tile([128, S + 2 * PAD + 2], F32R, tag="u")
                nc.vector.memset(u[:, 0:PAD].bitcast(F32), 0.0)
                nc.vector.memset(u[:, PAD + S:].bitcast(F32), 0.0)
                dwW = wdw.tile([128, KER, 128], F32R, tag="dww")
                nc.sync.dma_start(dwW[:], d["dwm"][cc].rearrange("t p m -> p t m"))
                for t in range(TC):
                    sl = slice(t * TN, (t + 1) * TN)
                    a_ps = ps_s.tile([128, TN], F32, tag="s")
                    accum(a_ps,
                          [(pw1[:, c, cc * 128:(cc + 1) * 128], h[:, c, sl])
                           for c in range(FC)],
                          bias=("pw1_b", cc) if bias_nz["pw1_b"] else None)
                    c_ps = ps_s.tile([128, TN], F32, tag="s")
                    accum(c_ps,
                          [(pw1[:, c, (ncc + cc) * 128:(ncc + cc + 1) * 128],
                            h[:, c, sl]) for c in range(FC)],
                          bias=("pw1_b", ncc + cc) if bias_nz["pw1_b"] else None)
                    sg = fsp.tile([128, TN], F32, tag="sg")
                    nc.scalar.activation(sg[:], c_ps[:], AF.Sigmoid)
                    nc.vector.tensor_tensor(u[:, PAD + t * TN:PAD + (t + 1) * TN],
                                            a_ps[:], sg[:], ALU.mult)
                for t in range(TC):
                    acc = ps_s.tile([128, TN], F32, tag="s")
                    accum(acc,
                          [(dwW[:, tap, :],
                            u[:, t * TN + tap:t * TN + tap + TN])
                           for tap in range(KER)],
                          bias=("dw_b", cc) if bias_nz["dw_b"] else None)
                    nc.scalar.activation(dvo[:, cc, t * TN:(t + 1) * TN], acc[:],
                                         AF.Silu)

            for t in range(TC):
                sl = slice(t * TN, (t + 1) * TN)
                for m in range(FC):
                    acc = ps_acc.tile([128, TN], F32, tag="acc")
                    accum(acc,
                          [(pw2[:, cc, m * 128:(m + 1) * 128], dvo[:, cc, sl])
                           for cc in range(ncc)],
                          bias=("pw2_b", m) if bias_nz["pw2_b"] else None)
                    nc.vector.tensor_tensor(xs[:, m, sl], acc[:],
                                            xs[:, m, sl].bitcast(F32), ALU.add)

        # ------------------------------------------------------------------
        def ff_pools(st):
            return (st.enter_context(tc.tile_pool(name="lnp", bufs=1)),
                    st.enter_context(tc.tile_pool(name="hp", bufs=1)),
                    st.enter_context(tc.tile_pool(name="wff", bufs=1)),
                    st.enter_context(tc.tile_pool(name="fsp", bufs=2)))

        def dbg_tap(i):
            if debug:
                nc.sync.dma_start(d[f"dbg{i}"].rearrange("(c p) n -> p c n", p=128),
                                  xs[:].bitcast(F32))

        for _rep in range(nreps):
            dbg = debug and _rep == nreps - 1
            with ExitStack() as st:
                ffn("ff1", ff_pools(st), dbg=dbg)
            if dbg:
                dbg_tap(1)
            with ExitStack() as st:
                pools = (st.enter_context(tc.tile_pool(name="lnp", bufs=1)),
                         st.enter_context(tc.tile_pool(name="hp", bufs=1)),
                         st.enter_context(tc.tile_pool(name="watt", bufs=1)),
                         st.enter_context(tc.tile_pool(name="attp", bufs=1)),
                         st.enter_context(tc.tile_pool(name="att2", bufs=2)),
                         st.enter_context(tc.tile_pool(name="att3", bufs=1)))
                attention(pools)
            if dbg:
                dbg_tap(2)
            with ExitStack() as st:
                pools = (st.enter_context(tc.tile_pool(name="lnp", bufs=1)),
                         st.enter_context(tc.tile_pool(name="hp", bufs=1)),
                         st.enter_context(tc.tile_pool(name="wconv", bufs=1)),
                         st.enter_context(tc.tile_pool(name="wdw", bufs=2)),
                         st.enter_context(tc.tile_pool(name="convp", bufs=1)),
                         st.enter_context(tc.tile_pool(name="up", bufs=3)),
                         st.enter_context(tc.tile_pool(name="fsp", bufs=2)))
                conv(pools)
            if dbg:
                dbg_tap(3)
            with ExitStack() as st:
                ffn("ff2", ff_pools(st))
            if dbg:
                dbg_tap(4)

        with ExitStack() as st:
            lnp = st.enter_context(tc.tile_pool(name="lnp", bufs=1))
            outt = spool.tile([128, FC, S], F32, tag="outt")
            layer_norm(outt, lnp)
            if bias_nz["fn"]:
                fg = cpool.tile([128, FC], F32, tag="fg")
                nc.sync.dma_start(fg[:], d["fn_g"].rearrange("(c p) -> p c", p=128))
                fb = cpool.tile([128, FC], F32, tag="fb")
                nc.sync.dma_start(fb[:], d["fn_b"].rearrange("(c p) -> p c", p=128))
                for c in range(FC):
                    nc.vector.tensor_scalar(outt[:, c, :], outt[:, c, :],
                                            fg[:, c:c + 1], fb[:, c:c + 1],
                                            ALU.mult, ALU.add)
        nc.sync.dma_start(d["out"].rearrange("(c p) n -> p c n", p=128), outt[:])

    nc.compile()
    return nc


# --------------------------------------------------------------------------
# SPMD execution (replicates bass2jax.run_bass_via_pjrt, reusable executable)
# --------------------------------------------------------------------------

class _Runner:
    def __init__(self, nc, n_cores=8):
        import jax
        from jax.sharding import Mesh, PartitionSpec
        from jax.experimental.shard_map import shard_map
        from concourse.bass2jax import (
            _bass_exec_p, install_neuronx_cc_hook, partition_id_tensor,
        )
        install_neuronx_cc_hook()
        self.jax = jax
        self.n_cores = n_cores
        partition_name = (nc.partition_id_tensor.name
                          if nc.partition_id_tensor else None)
        in_names, out_names, out_avals, zero_outs = [], [], [], []
        for alloc in nc.m.functions[0].allocations:
            if not isinstance(alloc, mybir.MemoryLocationSet):
                continue
            name = alloc.memorylocations[0].name
            if alloc.kind == "ExternalInput":
                if name != partition_name:
                    in_names.append(name)
            elif alloc.kind == "ExternalOutput":
                shape = tuple(alloc.tensor_shape)
                dtype = mybir.dt.np(alloc.dtype)
                out_names.append(name)
                out_avals.append(jax.core.ShapedArray(shape, dtype))
                zero_outs.append(np.zeros(shape, dtype))
        self.in_names, self.out_names = in_names, out_names
        self.out_avals, self.zero_outs = out_avals, zero_outs
        n_params, n_outs = len(in_names), len(out_avals)
        all_in = list(in_names) + list(out_names)
        if partition_name is not None:
            all_in.append(partition_name)
        donate = tuple(range(n_params, n_params + n_outs))

        def _body(*args):
            operands = list(args)
            if partition_name is not None:
                operands.append(partition_id_tensor())
            return tuple(_bass_exec_p.bind(
                *operands, out_avals=tuple(out_avals), in_names=tuple(all_in),
                out_names=tuple(out_names), lowering_input_output_aliases=(),
                sim_require_finite=True, sim_require_nnan=True, nc=nc))

        devices = jax.devices()[:n_cores]
        mesh = Mesh(np.asarray(devices), ("core",))
        in_specs = (PartitionSpec("core"),) * (n_params + n_outs)
        out_specs = (PartitionSpec("core"),) * n_outs
        self._fn = jax.jit(
            shard_map(_body, mesh=mesh, in_specs=in_specs, out_specs=out_specs,
                      check_rep=False),
            donate_argnums=donate, keep_unused=True)

    def concat_inputs(self, in_maps):
        n = self.n_cores
        per_core = [[np.asarray(m[name]) for name in self.in_names]
                    for m in in_maps]
        return [np.concatenate([per_core[c][i] for c in range(n)], axis=0)
                for i in range(len(self.in_names))]

    def run_concat(self, concat_in):
        n = self.n_cores
        zeros = [np.zeros((n * z.shape[0], *z.shape[1:]), z.dtype)
                 for z in self.zero_outs]
        out = self._fn(*concat_in, *zeros)
        self.jax.block_until_ready(out)
        return out

    def __call__(self, in_maps):
        out = self.run_concat(self.concat_inputs(in_maps))
        n = self.n_cores
        return [
            {name: np.asarray(out[i]).reshape(n, *self.out_avals[i].shape)[c]
             for i, name in enumerate(self.out_names)}
            for c in range(n)
        ]


def _get_runner(bias_nz, debug=False, nreps=1):
    key = (tuple(sorted(bias_nz.items())), debug, nreps)
    if key not in _CACHE:
        _CACHE[key] = _Runner(_build(bias_nz, debug=debug, nreps=nreps), 8)
    return _CACHE[key]


def _make_in_maps(inputs):
    p = _prep(inputs)
    x = np.asarray(inputs["x"], np.float32)
    bias_nz = {
        "ff1_bin": bool(np.any(p["ff1_bin"])), "ff1_bout": bool(np.any(p["ff1_bout"])),
        "ff2_bin": bool(np.any(p["ff2_bin"])), "ff2_bout": bool(np.any(p["ff2_bout"])),
        "bqkv": bool(np.any(p["bqkv"])), "bo": bool(np.any(p["bo"])),
        "pw1_b": bool(np.any(p["pw1_b"])), "dw_b": bool(np.any(p["dw_b"])),
        "pw2_b": bool(np.any(p["pw2_b"])),
        "fn": bool(np.any(p["fn_g"] != 1.0) or np.any(p["fn_b"])),
    }
    shared = {k: p[k] for k in
              ("ones", "onesrow", "ff1_win_t", "ff1_wout_t", "ff1_bin", "ff1_bout",
               "ff2_win_t", "ff2_wout_t", "ff2_bin", "ff2_bout",
               "wqkv_t", "bqkv", "wo_t", "bo", "pw1_t", "pw1_b", "dwm", "dw_b",
               "pw2_t", "pw2_b", "fn_g", "fn_b")}
    in_maps = []
    for b in range(B):
        m = dict(shared)
        m["x"] = np.ascontiguousarray(x[b].T)          # [512, 1024]
        in_maps.append(m)
    return in_maps, bias_nz


def kernel(**inputs):
    in_maps, bias_nz = _make_in_maps(inputs)
    runner = _get_runner(bias_nz)
    results = runner(in_maps)
    out = np.stack([results[b]["out"].T for b in range(B)], axis=0)
    return np.ascontiguousarray(out.astype(np.float32))


# revision 14
# speedup vs baseline: 59.2851x; 59.2851x over previous
"""ConformerBlock Trainium2 Bass kernel.

Sharding: data-parallel over batch (B=8) across the 8 NeuronCores; all
weights replicated per core; no collectives.

Per-core layout: feature-major residual stream xT [D=512, S=1024] in SBUF
as a [128, 4, 1024] tile. All matmuls run in float32r (TF32-like, ~1e-4
rel error, full PE rate at N=512). Host-side weight prep:
  - LayerNorm gamma/beta folded into the following linear's weights/bias
  - the 0.5 FFN scale folded into the FFN output weights
  - BatchNorm folded into the depthwise-conv weights/bias
  - depthwise grouped conv lowered to 31 taps x 8 channel-chunks of
    block-diagonal [128, 128] matrices applied as shifted accumulating
    matmuls in PSUM.
LN statistics use an all-ones [128,128] lhsT matmul, which yields the
per-token sums broadcast across all 128 partitions for free; softmax
normalization is deferred past the attention-value matmul (the value
matrix carries an extra ones column so PSUM row 64 accumulates the
softmax denominators), then applied via a K=1 broadcast matmul.
"""
import sys

sys.path.insert(0, '/opt/trn_rl_repo')

import numpy as np

import concourse.tile as tile
from concourse import bacc, mybir

F32 = mybir.dt.float32
F32R = mybir.dt.float32r
AF = mybir.ActivationFunctionType
ALU = mybir.AluOpType

D = 512            # model dim
S = 1024           # sequence length
B = 8              # batch (one element per core)
HEADS = 8
DH = 64            # head dim
FF_HID = 2048      # ffn hidden (per GLU half)
CONV_IN = 1024     # conv inner dim (per GLU half)
KER = 31
PAD = 15
LN_EPS = 1e-5
BN_EPS = 1e-5
TC = 2             # token chunks
TN = 512           # tokens per chunk
FC = 4             # feature chunks of D

_CACHE = {}


# --------------------------------------------------------------------------
# host-side weight preparation
# --------------------------------------------------------------------------

def _prep(inputs):
    g = {k: np.asarray(v, dtype=np.float64) for k, v in inputs.items()}
    p = {}

    def lin(w, ln_g, ln_b, bias):
        # y = ln_out @ W.T + bias with ln_out = xhat*g + b  =>
        # y = xhat @ (W*g).T + (W @ b + bias)
        return w * ln_g[None, :], w @ ln_b + bias

    for tag in ("ff1", "ff2"):
        w_in, b_in = lin(g[f"{tag}_win"], g[f"{tag}_g"], g[f"{tag}_b"],
                         g[f"{tag}_bin"])
        p[f"{tag}_win_t"] = np.ascontiguousarray(w_in.T, np.float32)         # [512, 4096]
        p[f"{tag}_bin"] = b_in.astype(np.float32)
        p[f"{tag}_wout_t"] = np.ascontiguousarray((0.5 * g[f"{tag}_wout"]).T,
                                                  np.float32)                # [2048, 512]
        p[f"{tag}_bout"] = (0.5 * g[f"{tag}_bout"]).astype(np.float32)

    wqkv, bqkv = lin(g["wqkv"], g["attn_g"], g["attn_b"], g["bqkv"])
    p["wqkv_t"] = np.ascontiguousarray(wqkv.T, np.float32)                   # [512, 1536]
    p["bqkv"] = bqkv.astype(np.float32)
    p["wo_t"] = np.ascontiguousarray(g["wo"].T, np.float32)                  # [512, 512]
    p["bo"] = g["bo"].astype(np.float32)

    pw1, pw1b = lin(g["pw1_w"][:, :, 0], g["conv_g"], g["conv_b"], g["pw1_b"])
    p["pw1_t"] = np.ascontiguousarray(pw1.T, np.float32)                     # [512, 2048]
    p["pw1_b"] = pw1b.astype(np.float32)

    # depthwise conv + BN fold -> block-diag matrices [8 cc][31 t][128 ki, 128 mo]
    scale = g["bn_g"] / np.sqrt(g["bn_v"] + BN_EPS)                          # [1024]
    dw = g["dw_w"] * scale[:, None, None]                                    # [1024, 8, 31]
    dwb = (g["dw_b"] - g["bn_m"]) * scale + g["bn_b"]                        # [1024]
    dwm = np.zeros((8, KER, 128, 128), np.float64)
    o = np.arange(128)
    grp = (o // 8) * 8
    for j in range(8):
        ki = grp + j
        for cc in range(8):
            dwm[cc, :, ki, o] = dw[cc * 128 + o, j, :]   # LHS adv-index -> [128, 31]
    p["dwm"] = np.ascontiguousarray(dwm, np.float32)
    p["dw_b"] = dwb.astype(np.float32)

    p["pw2_t"] = np.ascontiguousarray(g["pw2_w"][:, :, 0].T, np.float32)     # [1024, 512]
    p["pw2_b"] = g["pw2_b"].astype(np.float32)

    p["fn_g"] = g["fn_g"].astype(np.float32)
    p["fn_b"] = g["fn_b"].astype(np.float32)

    p["ones"] = np.ones((128, 128), np.float32)
    p["onesrow"] = np.ones((1, TN), np.float32)
    return p


# --------------------------------------------------------------------------
# device program
# --------------------------------------------------------------------------

def _build(bias_nz, debug=False, nreps=1, phases=("ff1", "attn", "conv", "ff2")):
    """bias_nz: dict name->bool; device-side bias adds are emitted only for
    entries that are True (the rest are exactly zero in the input arrays)."""
    nc = bacc.Bacc("TRN2", target_bir_lowering=False, debug=False)

    d = {}
    d["x"] = nc.dram_tensor("x", [D, S], F32R, kind="ExternalInput").ap()
    d["ones"] = nc.dram_tensor("ones", [128, 128], F32R, kind="ExternalInput").ap()
    d["onesrow"] = nc.dram_tensor("onesrow", [1, TN], F32R, kind="ExternalInput").ap()
    for tag in ("ff1", "ff2"):
        d[f"{tag}_win_t"] = nc.dram_tensor(f"{tag}_win_t", [D, 2 * FF_HID], F32R,
                                           kind="ExternalInput").ap()
        d[f"{tag}_wout_t"] = nc.dram_tensor(f"{tag}_wout_t", [FF_HID, D], F32R,
                                            kind="ExternalInput").ap()
        d[f"{tag}_bin"] = nc.dram_tensor(f"{tag}_bin", [2 * FF_HID], F32R,
                                         kind="ExternalInput").ap()
        d[f"{tag}_bout"] = nc.dram_tensor(f"{tag}_bout", [D], F32R,
                                          kind="ExternalInput").ap()
    d["wqkv_t"] = nc.dram_tensor("wqkv_t", [D, 3 * D], F32R, kind="ExternalInput").ap()
    d["bqkv"] = nc.dram_tensor("bqkv", [3 * D], F32R, kind="ExternalInput").ap()
    d["wo_t"] = nc.dram_tensor("wo_t", [D, D], F32R, kind="ExternalInput").ap()
    d["bo"] = nc.dram_tensor("bo", [D], F32R, kind="ExternalInput").ap()
    d["pw1_t"] = nc.dram_tensor("pw1_t", [D, 2 * CONV_IN], F32R,
                                kind="ExternalInput").ap()
    d["pw1_b"] = nc.dram_tensor("pw1_b", [2 * CONV_IN], F32R, kind="ExternalInput").ap()
    d["dwm"] = nc.dram_tensor("dwm", [8, KER, 128, 128], F32R,
                              kind="ExternalInput").ap()
    d["dw_b"] = nc.dram_tensor("dw_b", [CONV_IN], F32R, kind="ExternalInput").ap()
    d["pw2_t"] = nc.dram_tensor("pw2_t", [CONV_IN, D], F32R, kind="ExternalInput").ap()
    d["pw2_b"] = nc.dram_tensor("pw2_b", [D], F32R, kind="ExternalInput").ap()
    d["fn_g"] = nc.dram_tensor("fn_g", [D], F32, kind="ExternalInput").ap()
    d["fn_b"] = nc.dram_tensor("fn_b", [D], F32, kind="ExternalInput").ap()
    d["out"] = nc.dram_tensor("out", [D, S], F32, kind="ExternalOutput").ap()
    if debug:
        for i in range(1, 5):
            d[f"dbg{i}"] = nc.dram_tensor(f"dbg{i}", [D, S], F32,
                                          kind="ExternalOutput").ap()
        d["dbgh"] = nc.dram_tensor("dbgh", [D, S], F32, kind="ExternalOutput").ap()

    from contextlib import ExitStack
    with tile.TileContext(nc) as tc, ExitStack() as ctx:
        cpool = ctx.enter_context(tc.tile_pool(name="cpool", bufs=1))
        spool = ctx.enter_context(tc.tile_pool(name="spool", bufs=1))
        bp = ctx.enter_context(tc.tile_pool(name="bp", bufs=1))
        ps_acc = ctx.enter_context(tc.tile_pool(name="ps_acc", bufs=4, space="PSUM"))
        ps_s = ctx.enter_context(tc.tile_pool(name="ps_s", bufs=3, space="PSUM"))

        ones = cpool.tile([128, 128], F32R)
        nc.sync.dma_start(ones[:], d["ones"])

        any_bias = any(v for k, v in bias_nz.items() if k != "fn")
        onesrow = None
        if any_bias:
            onesrow = cpool.tile([1, TN], F32R, tag="onesrow")
            nc.sync.dma_start(onesrow[:], d["onesrow"])

        xs = spool.tile([128, FC, S], F32R)
        nc.sync.dma_start(xs[:], d["x"].rearrange("(c p) n -> p c n", p=128))

        brow = {}
        for name, width in (("ff1_bin", 2 * FF_HID), ("ff1_bout", D),
                            ("ff2_bin", 2 * FF_HID), ("ff2_bout", D),
                            ("bqkv", 3 * D), ("bo", D), ("pw1_b", 2 * CONV_IN),
                            ("dw_b", CONV_IN), ("pw2_b", D)):
            if bias_nz.get(name):
                t = bp.tile([1, width], F32R, tag=name)
                nc.sync.dma_start(t[:], d[name][None, :])
                brow[name] = t

        def accum(acc, terms, bias=None, n=TN):
            """PSUM accumulation: matmuls over `terms` [(lhsT, rhs)...] plus an
            optional per-partition bias via a K=1 matmul; sets start/stop."""
            last = len(terms) - 1 + (1 if bias else 0)
            i = 0
            for l, r in terms:
                nc.tensor.matmul(acc[:], l, r, start=(i == 0), stop=(i == last))
                i += 1
            if bias:
                name, mo = bias
                nc.tensor.matmul(acc[:],
                                 brow[name][:, mo * 128:(mo + 1) * 128],
                                 onesrow[:, 0:n],
                                 start=False, stop=True)

        # ------------------------------------------------------------------
        def layer_norm(h_out, lnp):
            for t in range(TC):
                sl = slice(t * TN, (t + 1) * TN)
                bc_s = ps_s.tile([128, TN], F32, tag="s")
                accum(bc_s, [(ones[:], xs[:, c, sl]) for c in range(FC)])
                xsq = lnp.tile([128, FC, TN], F32R, tag="xsq")
                for c in range(FC):
                    nc.scalar.activation(xsq[:, c, :], xs[:, c, sl].bitcast(F32),
                                         AF.Square)
                bc_q = ps_s.tile([128, TN], F32, tag="s")
                accum(bc_q, [(ones[:], xsq[:, c, :]) for c in range(FC)])
                mu = lnp.tile([128, TN], F32, tag="mu")
                nc.vector.tensor_scalar(mu[:], bc_s[:], 1.0 / D, None, ALU.mult)
                ve = lnp.tile([128, TN], F32, tag="ve")
                nc.vector.tensor_scalar(ve[:], bc_q[:], 1.0 / D, LN_EPS,
                                        ALU.mult, ALU.add)
                m2 = lnp.tile([128, TN], F32, tag="m2")
                nc.vector.tensor_tensor(m2[:], mu[:], mu[:], ALU.mult)
                nc.vector.tensor_tensor(ve[:], ve[:], m2[:], ALU.subtract)
                nc.scalar.activation(m2[:], ve[:], AF.Ln)
                rsig = lnp.tile([128, TN], F32, tag="rsig")
                nc.scalar.activation(rsig[:], m2[:], AF.Exp, scale=-0.5)
                for c in range(FC):
                    dd = lnp.tile([128, TN], F32, tag="dd")
                    nc.vector.tensor_tensor(dd[:], xs[:, c, sl].bitcast(F32),
                                            mu[:], ALU.subtract)
                    nc.vector.tensor_tensor(h_out[:, c, sl], dd[:], rsig[:],
                                            ALU.mult)

        # ------------------------------------------------------------------
        def ffn(tag, pools, dbg=False):
            lnp, hpool, wff, fsp = pools
            h = hpool.tile([128, FC, S], F32R, tag="h")
            layer_norm(h, lnp)
            if dbg:
                nc.sync.dma_start(d["dbgh"].rearrange("(c p) n -> p c n", p=128),
                                  h[:].bitcast(F32))
            w_in = wff.tile([128, FC, 2 * FF_HID], F32R, tag="wffin")
            nc.sync.dma_start(w_in[:],
                              d[f"{tag}_win_t"].rearrange("(c p) m -> p c m", p=128))
            nh = FF_HID // 128  # 16
            w_out = wff.tile([128, nh, D], F32R, tag="wffout")
            nc.sync.dma_start(w_out[:],
                              d[f"{tag}_wout_t"].rearrange("(c p) m -> p c m", p=128))
            b_in = f"{tag}_bin"
            b_out = f"{tag}_bout"
            for t in range(TC):
                sl = slice(t * TN, (t + 1) * TN)
                accs = []
                for m in range(FC):
                    acc_t = ps_acc.tile([128, TN], F32, tag="acc")
                    accs.append(acc_t)
                for hc in range(nh):
                    a_ps = ps_s.tile([128, TN], F32, tag="s")
                    accum(a_ps,
                          [(w_in[:, c, hc * 128:(hc + 1) * 128], h[:, c, sl])
                           for c in range(FC)],
                          bias=(b_in, hc) if bias_nz[b_in] else None)
                    c_ps = ps_s.tile([128, TN], F32, tag="s")
                    accum(c_ps,
                          [(w_in[:, c, (nh + hc) * 128:(nh + hc + 1) * 128],
                            h[:, c, sl]) for c in range(FC)],
                          bias=(b_in, nh + hc) if bias_nz[b_in] else None)
                    a_sb = fsp.tile([128, TN], F32, tag="asb")
                    nc.scalar.activation(a_sb[:], a_ps[:], AF.Silu)
                    g_sb = fsp.tile([128, TN], F32R, tag="gsb")
                    nc.vector.tensor_tensor(g_sb[:], a_sb[:], c_ps[:], ALU.mult)
                    for m in range(FC):
                        is_last = hc == nh - 1 and not bias_nz[b_out]
                        nc.tensor.matmul(accs[m][:],
                                         w_out[:, hc, m * 128:(m + 1) * 128],
                                         g_sb[:],
                                         start=(hc == 0), stop=is_last)
                for m in range(FC):
                    if bias_nz[b_out]:
                        nc.tensor.matmul(accs[m][:],
                                         brow[b_out][:, m * 128:(m + 1) * 128],
                                         onesrow[:, 0:TN], start=False, stop=True)
                    nc.vector.tensor_tensor(xs[:, m, sl], accs[m][:],
                                            xs[:, m, sl].bitcast(F32), ALU.add)

        # ------------------------------------------------------------------
        def attention(pools):
            lnp, hpool, watt, attp, att2, att3 = pools
            h = hpool.tile([128, FC, S], F32R, tag="h")
            layer_norm(h, lnp)
            wqkv = watt.tile([128, FC, 3 * D], F32R, tag="wqkv")
            nc.sync.dma_start(wqkv[:],
                              d["wqkv_t"].rearrange("(c p) m -> p c m", p=128))
            wo = watt.tile([128, FC, D], F32R, tag="wo")
            nc.sync.dma_start(wo[:], d["wo_t"].rearrange("(c p) m -> p c m", p=128))
            bvq = None
            if bias_nz["bqkv"]:
                bvq = bp.tile([128, FC], F32, tag="bvq")
                nc.sync.dma_start(bvq[:],
                                  d["bqkv"][2 * D:3 * D].rearrange("(c p) -> p c",
                                                                   p=128))

            q_sb = attp.tile([128, FC, S], F32R, tag="q")
            k_sb = attp.tile([128, FC, S], F32R, tag="k")
            for fc in range(FC):
                for t in range(TC):
                    sl = slice(t * TN, (t + 1) * TN)
                    for which, base in (("q", 0), ("k", D)):
                        pp = ps_s.tile([128, TN], F32, tag="s")
                        mo = base // 128 + fc
                        accum(pp,
                              [(wqkv[:, c, mo * 128:(mo + 1) * 128], h[:, c, sl])
                               for c in range(FC)],
                              bias=("bqkv", mo) if bias_nz["bqkv"] else None)
                        dst = q_sb if which == "q" else k_sb
                        nc.scalar.activation(dst[:, fc, sl], pp[:], AF.Copy)

            # v token-major with ones column at index 64 per head
            vaug = attp.tile([128, 8, HEADS, 66], F32R, tag="vaug")
            nc.sync.dma_start(
                vaug[:, :, :, 64:65],
                d["ones"][:, 0:64].rearrange("p (a b) -> p a b", a=8).unsqueeze(3))
            for kc in range(8):
                v_ps = ps_s.tile([128, D], F32, tag="s")
                terms = [(h[:, c, kc * 128:(kc + 1) * 128], wqkv[:, c, 2 * D:3 * D])
                         for c in range(FC)]
                last = len(terms) - 1 + (1 if bias_nz["bqkv"] else 0)
                for i, (l, r) in enumerate(terms):
                    nc.tensor.matmul(v_ps[:], l, r, start=(i == 0),
                                     stop=(i == last))
                if bias_nz["bqkv"]:
                    nc.tensor.matmul(v_ps[:], ones[0:1, :],
                                     brow["bqkv"][:, 2 * D:3 * D],
                                     start=False, stop=True)
                nc.scalar.activation(
                    vaug[:, kc, :, 0:64],
                    v_ps[:].rearrange("p (h e) -> p h e", h=HEADS), AF.Copy)

            o_fm = attp.tile([128, FC, S], F32R, tag="ofm")
            for hd in range(HEADS):
                hb = (hd % 2) * 64
                hc = hd // 2
                for t in range(TC):
                    sl = slice(t * TN, (t + 1) * TN)
                    e_sb = att2.tile([128, 8, TN], F32R, tag="esb")
                    o_ps = ps_acc.tile([65, TN], F32, tag="acc")
                    for kc in range(8):
                        s_ps = ps_s.tile([128, TN], F32, tag="s")
                        nc.tensor.matmul(s_ps[:],
                                         k_sb[hb:hb + 64, hc, kc * 128:(kc + 1) * 128],
                                         q_sb[hb:hb + 64, hc, sl],
                                         start=True, stop=True)
                        nc.scalar.activation(e_sb[:, kc, :], s_ps[:], AF.Exp,
                                             scale=float(DH) ** -0.5)
                        nc.tensor.matmul(o_ps[:],
                                         vaug[:, kc, hd, 0:65],
                                         e_sb[:, kc, :],
                                         start=(kc == 0), stop=(kc == 7))
                    rows = att3.tile([1, 3, TN], F32, tag="rows")
                    nc.scalar.activation(rows[:, 2, :], o_ps[64:65, :], AF.Copy)
                    nc.vector.reciprocal_approx_accurate(
                        rows[:, 0, :], rows[:, 2, :], rows[:, 1, :])
                    rrow_r = att3.tile([1, TN], F32R, tag="rrowr")
                    nc.scalar.activation(rrow_r[:], rows[:, 0, :], AF.Copy)
                    bc_ps = ps_s.tile([64, TN], F32, tag="s")
                    nc.tensor.matmul(bc_ps[:], ones[0:1, 0:64], rrow_r[:],
                                     start=True, stop=True)
                    bc_sb = att3.tile([64, TN], F32, tag="bcsb")
                    nc.scalar.activation(bc_sb[:], bc_ps[:], AF.Copy)
                    nc.vector.tensor_tensor(o_fm[hb:hb + 64, hc, sl],
                                            o_ps[0:64, :], bc_sb[:], ALU.mult)
                    if bias_nz["bqkv"]:
                        # + v bias (softmax weights sum to one)
                        nc.vector.tensor_scalar(
                            o_fm[hb:hb + 64, hc, sl],
                            o_fm[hb:hb + 64, hc, sl].bitcast(F32),
                            bvq[hb:hb + 64, hc:hc + 1], None, ALU.add)

            for t in range(TC):
                sl = slice(t * TN, (t + 1) * TN)
                for m in range(FC):
                    acc = ps_acc.tile([128, TN], F32, tag="acc")
                    accum(acc,
                          [(wo[:, c, m * 128:(m + 1) * 128], o_fm[:, c, sl])
                           for c in range(FC)],
                          bias=("bo", m) if bias_nz["bo"] else None)
                    nc.vector.tensor_tensor(xs[:, m, sl], acc[:],
                                            xs[:, m, sl].bitcast(F32), ALU.add)

        # ------------------------------------------------------------------
        def conv(pools):
            lnp, hpool, wconv, wdw, convp, up, fsp = pools
            h = hpool.tile([128, FC, S], F32R, tag="h")
            layer_norm(h, lnp)
            pw1 = wconv.tile([128, FC, 2 * CONV_IN], F32R, tag="pw1")
            nc.sync.dma_start(pw1[:],
                              d["pw1_t"].rearrange("(c p) m -> p c m", p=128))
            ncc = CONV_IN // 128  # 8
            pw2 = wconv.tile([128, ncc, D], F32R, tag="pw2")
            nc.sync.dma_start(pw2[:],
                              d["pw2_t"].rearrange("(c p) m -> p c m", p=128))

            dvo = convp.tile([128, ncc, S], F32R, tag="dvo")
            for cc in range(ncc):
                u = up.tile([128, S + 2 * PAD + 2], F32R, tag="u")
                nc.vector.memset(u[:, 0:PAD].bitcast(F32), 0.0)
                nc.vector.memset(u[:, PAD + S:].bitcast(F32), 0.0)
                dwW = wdw.tile([128, KER, 128], F32R, tag="dww")
                nc.sync.dma_start(dwW[:], d["dwm"][cc].rearrange("t p m -> p t m"))
                for t in range(TC):
                    sl = slice(t * TN, (t + 1) * TN)
                    a_ps = ps_s.tile([128, TN], F32, tag="s")
                    accum(a_ps,
                          [(pw1[:, c, cc * 128:(cc + 1) * 128], h[:, c, sl])
                           for c in range(FC)],
                          bias=("pw1_b", cc) if bias_nz["pw1_b"] else None)
                    c_ps = ps_s.tile([128, TN], F32, tag="s")
                    accum(c_ps,
                          [(pw1[:, c, (ncc + cc) * 128:(ncc + cc + 1) * 128],
                            h[:, c, sl]) for c in range(FC)],
                          bias=("pw1_b", ncc + cc) if bias_nz["pw1_b"] else None)
                    sg = fsp.tile([128, TN], F32, tag="sg")
                    nc.scalar.activation(sg[:], c_ps[:], AF.Sigmoid)
                    nc.vector.tensor_tensor(u[:, PAD + t * TN:PAD + (t + 1) * TN],
                                            a_ps[:], sg[:], ALU.mult)
                for t in range(TC):
                    acc = ps_s.tile([128, TN], F32, tag="s")
                    accum(acc,
                          [(dwW[:, tap, :],
                            u[:, t * TN + tap:t * TN + tap + TN])
                           for tap in range(KER)],
                          bias=("dw_b", cc) if bias_nz["dw_b"] else None)
                    nc.scalar.activation(dvo[:, cc, t * TN:(t + 1) * TN], acc[:],
                                         AF.Silu)

            for t in range(TC):
                sl = slice(t * TN, (t + 1) * TN)
                for m in range(FC):
                    acc = ps_acc.tile([128, TN], F32, tag="acc")
                    accum(acc,
                          [(pw2[:, cc, m * 128:(m + 1) * 128], dvo[:, cc, sl])
                           for cc in range(ncc)],
                          bias=("pw2_b", m) if bias_nz["pw2_b"] else None)
                    nc.vector.tensor_tensor(xs[:, m, sl], acc[:],
                                            xs[:, m, sl].bitcast(F32), ALU.add)

        # ------------------------------------------------------------------
        def ff_pools(st):
            return (st.enter_context(tc.tile_pool(name="lnp", bufs=1)),
                    st.enter_context(tc.tile_pool(name="hp", bufs=1)),
                    st.enter_context(tc.tile_pool(name="wff", bufs=1)),
                    st.enter_context(tc.tile_pool(name="fsp", bufs=2)))

        def dbg_tap(i):
            if debug:
                nc.sync.dma_start(d[f"dbg{i}"].rearrange("(c p) n -> p c n", p=128),
                                  xs[:].bitcast(F32))

        for _rep in range(nreps):
            dbg = debug and _rep == nreps - 1
            if "ff1" in phases:
                with ExitStack() as st:
                    ffn("ff1", ff_pools(st), dbg=dbg)
            if dbg:
                dbg_tap(1)
            if "attn" in phases:
                with ExitStack() as st:
                    pools = (st.enter_context(tc.tile_pool(name="lnp", bufs=1)),
                         st.enter_context(tc.tile_pool(name="hp", bufs=1)),
                         st.enter_context(tc.tile_pool(name="watt", bufs=1)),
                         st.enter_context(tc.tile_pool(name="attp", bufs=1)),
                         st.enter_context(tc.tile_pool(name="att2", bufs=2)),
                         st.enter_context(tc.tile_pool(name="att3", bufs=1)))
                    attention(pools)
            if dbg:
                dbg_tap(2)
            if "conv" in phases:
                with ExitStack() as st:
                    pools = (st.enter_context(tc.tile_pool(name="lnp", bufs=1)),
                             st.enter_context(tc.tile_pool(name="hp", bufs=1)),
                             st.enter_context(tc.tile_pool(name="wconv", bufs=1)),
                             st.enter_context(tc.tile_pool(name="wdw", bufs=2)),
                             st.enter_context(tc.tile_pool(name="convp", bufs=1)),
                             st.enter_context(tc.tile_pool(name="up", bufs=3)),
                             st.enter_context(tc.tile_pool(name="fsp", bufs=2)))
                    conv(pools)
            if dbg:
                dbg_tap(3)
            if "ff2" in phases:
                with ExitStack() as st:
                    ffn("ff2", ff_pools(st))
            if dbg:
                dbg_tap(4)

        with ExitStack() as st:
            lnp = st.enter_context(tc.tile_pool(name="lnp", bufs=1))
            outt = spool.tile([128, FC, S], F32, tag="outt")
            layer_norm(outt, lnp)
            if bias_nz["fn"]:
                fg = cpool.tile([128, FC], F32, tag="fg")
                nc.sync.dma_start(fg[:], d["fn_g"].rearrange("(c p) -> p c", p=128))
                fb = cpool.tile([128, FC], F32, tag="fb")
                nc.sync.dma_start(fb[:], d["fn_b"].rearrange("(c p) -> p c", p=128))
                for c in range(FC):
                    nc.vector.tensor_scalar(outt[:, c, :], outt[:, c, :],
                                            fg[:, c:c + 1], fb[:, c:c + 1],
                                            ALU.mult, ALU.add)
        nc.sync.dma_start(d["out"].rearrange("(c p) n -> p c n", p=128), outt[:])

    nc.compile()
    return nc


# --------------------------------------------------------------------------
# SPMD execution (replicates bass2jax.run_bass_via_pjrt, reusable executable)
# --------------------------------------------------------------------------

class _Runner:
    def __init__(self, nc, n_cores=8):
        import jax
        from jax.sharding import Mesh, PartitionSpec
        from jax.experimental.shard_map import shard_map
        from concourse.bass2jax import (
            _bass_exec_p, install_neuronx_cc_hook, partition_id_tensor,
        )
        install_neuronx_cc_hook()
        self.jax = jax
        self.n_cores = n_cores
        partition_name = (nc.partition_id_tensor.name
                          if nc.partition_id_tensor else None)
        in_names, out_names, out_avals, zero_outs = [], [], [], []
        for alloc in nc.m.functions[0].allocations:
            if not isinstance(alloc, mybir.MemoryLocationSet):
                continue
            name = alloc.memorylocations[0].name
            if alloc.kind == "ExternalInput":
                if name != partition_name:
                    in_names.append(name)
            elif alloc.kind == "ExternalOutput":
                shape = tuple(alloc.tensor_shape)
                dtype = mybir.dt.np(alloc.dtype)
                out_names.append(name)
                out_avals.append(jax.core.ShapedArray(shape, dtype))
                zero_outs.append(np.zeros(shape, dtype))
        self.in_names, self.out_names = in_names, out_names
        self.out_avals, self.zero_outs = out_avals, zero_outs
        n_params, n_outs = len(in_names), len(out_avals)
        all_in = list(in_names) + list(out_names)
        if partition_name is not None:
            all_in.append(partition_name)
        donate = tuple(range(n_params, n_params + n_outs))

        def _body(*args):
            operands = list(args)
            if partition_name is not None:
                operands.append(partition_id_tensor())
            return tuple(_bass_exec_p.bind(
                *operands, out_avals=tuple(out_avals), in_names=tuple(all_in),
                out_names=tuple(out_names), lowering_input_output_aliases=(),
                sim_require_finite=True, sim_require_nnan=True, nc=nc))

        devices = jax.devices()[:n_cores]
        mesh = Mesh(np.asarray(devices), ("core",))
        in_specs = (PartitionSpec("core"),) * (n_params + n_outs)
        out_specs = (PartitionSpec("core"),) * n_outs
        self._fn = jax.jit(
            shard_map(_body, mesh=mesh, in_specs=in_specs, out_specs=out_specs,
                      check_rep=False),
            donate_argnums=donate, keep_unused=True)

    def concat_inputs(self, in_maps):
        n = self.n_cores
        per_core = [[np.asarray(m[name]) for name in self.in_names]
                    for m in in_maps]
        return [np.concatenate([per_core[c][i] for c in range(n)], axis=0)
                for i in range(len(self.in_names))]

    def run_concat(self, concat_in):
        n = self.n_cores
        zeros = [np.zeros((n * z.shape[0], *z.shape[1:]), z.dtype)
                 for z in self.zero_outs]
        out = self._fn(*concat_in, *zeros)
        self.jax.block_until_ready(out)
        return out

    def __call__(self, in_maps):
        out = self.run_concat(self.concat_inputs(in_maps))
        n = self.n_cores
        return [
            {name: np.asarray(out[i]).reshape(n, *self.out_avals[i].shape)[c]
             for i, name in enumerate(self.out_names)}
            for c in range(n)
        ]


def _get_runner(bias_nz, debug=False, nreps=1,
                phases=("ff1", "attn", "conv", "ff2")):
    key = (tuple(sorted(bias_nz.items())), debug, nreps, tuple(phases))
    if key not in _CACHE:
        _CACHE[key] = _Runner(
            _build(bias_nz, debug=debug, nreps=nreps, phases=phases), 8)
    return _CACHE[key]


def _make_in_maps(inputs):
    p = _prep(inputs)
    x = np.asarray(inputs["x"], np.float32)
    bias_nz = {
        "ff1_bin": bool(np.any(p["ff1_bin"])), "ff1_bout": bool(np.any(p["ff1_bout"])),
        "ff2_bin": bool(np.any(p["ff2_bin"])), "ff2_bout": bool(np.any(p["ff2_bout"])),
        "bqkv": bool(np.any(p["bqkv"])), "bo": bool(np.any(p["bo"])),
        "pw1_b": bool(np.any(p["pw1_b"])), "dw_b": bool(np.any(p["dw_b"])),
        "pw2_b": bool(np.any(p["pw2_b"])),
        "fn": bool(np.any(p["fn_g"] != 1.0) or np.any(p["fn_b"])),
    }
    shared = {k: p[k] for k in
              ("ones", "onesrow", "ff1_win_t", "ff1_wout_t", "ff1_bin", "ff1_bout",
               "ff2_win_t", "ff2_wout_t", "ff2_bin", "ff2_bout",
               "wqkv_t", "bqkv", "wo_t", "bo", "pw1_t", "pw1_b", "dwm", "dw_b",
               "pw2_t", "pw2_b", "fn_g", "fn_b")}
    in_maps = []
    for b in range(B):
        m = dict(shared)
        m["x"] = np.ascontiguousarray(x[b].T)          # [512, 1024]
        in_maps.append(m)
    return in_maps, bias_nz


def kernel(**inputs):
    in_maps, bias_nz = _make_in_maps(inputs)
    runner = _get_runner(bias_nz)
    results = runner(in_maps)
    out = np.stack([results[b]["out"].T for b in range(B)], axis=0)
    return np.ascontiguousarray(out.astype(np.float32))
